# revision 1
# baseline (speedup 1.0000x reference)
"""Double-centering kernel for Trainium2 (Bass/Tile), 8-core data parallel.

Computes T = -0.5 * (D - row_mean - col_mean + glob_mean) for
D: [256, 512, 512] f32, sharding the batch dim across 8 NeuronCores
(32 matrices per core, no cross-core communication).

Per-core layout: PAIRS of [512, 512] matrices are viewed as one
[128, 4096] SBUF tile (matrix m in cols m*2048..; partition p holds its
rows 4p..4p+3), so every DMA is one fully contiguous 2 MiB transfer.

Per-pair dataflow (engine balance is the point — DMA is the roofline):
  SP:     2 MiB load -> in_t (HWDGE ring)
  GPSIMD: S2 = c01+c23, S = S2a+S2b per matrix (partial col sums)
          2 MiB store <- v (SWDGE, separate DMA path from loads)
  PE:     C0 = ones[128,128]^T @ S -> PSUM    (col sums bcast, per matrix)
  ACT:    v_c = -0.5*D_c (accum_out a_c = -0.5*rowsum_c), separate v tile
          Csc = C0/1024 (accum_out gsum = g/1024)  (= 0.5*col_mean)
  DVE:    rowterm = -(a+gsum)/512             (= 0.5*row_mean - 0.5*glob_mean)
          v_c = (v_c + rowterm_c) + Csc       (scalar_tensor_tensor, in place)

in_t is only read by S2 and the v-pass, so its slot recycles early and
the load pipeline stays deep; v carries the tail (stt -> store).
"""

from contextlib import ExitStack

import numpy as np

import concourse.bacc as bacc
import concourse.tile as tile
from concourse import mybir
from concourse.bass_utils import run_bass_kernel_spmd

N_CORES = 8
B = 256
N = 512
B_LOC = B // N_CORES  # 32 matrices per core
PAIR = 2
N_PAIRS = B_LOC // PAIR  # 16 DMA pairs per core
P = 128
CHUNKS = N // P  # 4
FREE = CHUNKS * N  # 2048 elems per partition per matrix
PFREE = PAIR * FREE  # 4096 per pair tile

_COMPILED = None
LAST_RESULTS = None  # BassKernelResults of the most recent run (for test harness)


def _build():
    nc = bacc.Bacc("TRN2", target_bir_lowering=False, debug=False)
    d_in = nc.dram_tensor("d_in", [N_PAIRS, P, PFREE], mybir.dt.float32,
                          kind="ExternalInput")
    t_out = nc.dram_tensor("t_out", [N_PAIRS, P, PFREE], mybir.dt.float32,
                           kind="ExternalOutput")
    f32 = mybir.dt.float32

    with tile.TileContext(nc) as tc, ExitStack() as ctx:
        singles = ctx.enter_context(tc.tile_pool(name="singles", bufs=1))
        in_pool = ctx.enter_context(tc.tile_pool(name="in", bufs=4))
        v_pool = ctx.enter_context(tc.tile_pool(name="v", bufs=3))
        s2_pool = ctx.enter_context(tc.tile_pool(name="s2", bufs=2))
        s_pool = ctx.enter_context(tc.tile_pool(name="s", bufs=2))
        csc_pool = ctx.enter_context(tc.tile_pool(name="csc", bufs=3))
        small = ctx.enter_context(tc.tile_pool(name="small", bufs=6))
        psum = ctx.enter_context(tc.tile_pool(name="psum", bufs=4, space="PSUM"))

        ones_kk = singles.tile([P, P], f32)
        nc.vector.memset(ones_kk[:], 1.0)

        for bp in range(N_PAIRS):
            in_t = in_pool.tile([P, PFREE], f32)
            nc.sync.dma_start(out=in_t[:], in_=d_in[bp])

            # Partial column sums per matrix, both matrices in one op pair:
            # view pair as [128, 2, 2048]; halves add -> S2 [128, 2, 1024].
            in3 = in_t[:].rearrange("p (m f) -> p m f", m=PAIR)
            s2 = s2_pool.tile([P, PAIR, 2 * N], f32)
            nc.gpsimd.tensor_add(out=s2[:], in0=in3[:, :, :2 * N],
                                 in1=in3[:, :, 2 * N:])
            s = s_pool.tile([P, PAIR, N], f32)
            nc.gpsimd.tensor_add(out=s[:], in0=s2[:, :, :N], in1=s2[:, :, N:])

            # Column sums broadcast to all 128 partitions via all-ones
            # matmul, one per matrix (N<=512 fp32 per PSUM bank).
            csc = csc_pool.tile([P, PAIR, N], f32)
            gsum = small.tile([P, PAIR], f32)
            v = v_pool.tile([P, PFREE], f32)
            a = small.tile([P, PAIR * CHUNKS], f32)
            for m in range(PAIR):
                c0 = psum.tile([P, N], f32)
                nc.tensor.matmul(out=c0[:], lhsT=ones_kk[:], rhs=s[:, m, :],
                                 start=True, stop=True)

                # v_c = -0.5 * D_c (ACT); a_c = -0.5 * rowsum_c.
                for c in range(CHUNKS):
                    sl = slice(m * FREE + c * N, m * FREE + (c + 1) * N)
                    k = m * CHUNKS + c
                    nc.scalar.activation(out=v[:, sl], in_=in_t[:, sl],
                                         func=mybir.ActivationFunctionType.Copy,
                                         bias=0.0, scale=-0.5,
                                         accum_out=a[:, k:k + 1])

                # Csc = 0.5*col_mean (SBUF); gsum = g/1024 per partition.
                nc.scalar.activation(out=csc[:, m, :], in_=c0[:],
                                     func=mybir.ActivationFunctionType.Copy,
                                     bias=0.0, scale=1.0 / 1024.0,
                                     accum_out=gsum[:, m:m + 1])

            # rowterm = 0.5*row_mean - 0.5*glob_mean = -(a + gsum)/512,
            # per matrix (gsum differs between the two matrices).
            rowterm = small.tile([P, PAIR * CHUNKS], f32)
            for m in range(PAIR):
                ksl = slice(m * CHUNKS, (m + 1) * CHUNKS)
                nc.vector.tensor_scalar(out=rowterm[:, ksl], in0=a[:, ksl],
                                        scalar1=gsum[:, m:m + 1],
                                        scalar2=-1.0 / 512.0,
                                        op0=mybir.AluOpType.add,
                                        op1=mybir.AluOpType.mult)

            # out_c = (v_c + rowterm_c) + Csc, fused and in place.
            for m in range(PAIR):
                for c in range(CHUNKS):
                    sl = slice(m * FREE + c * N, m * FREE + (c + 1) * N)
                    k = m * CHUNKS + c
                    nc.vector.scalar_tensor_tensor(out=v[:, sl],
                                                   in0=v[:, sl],
                                                   scalar=rowterm[:, k:k + 1],
                                                   in1=csc[:, m, :],
                                                   op0=mybir.AluOpType.add,
                                                   op1=mybir.AluOpType.add)

            nc.gpsimd.dma_start(out=t_out[bp], in_=v[:])

    nc.compile()
    return nc


def _get_nc():
    global _COMPILED
    if _COMPILED is None:
        _COMPILED = _build()
    return _COMPILED


def kernel(D: np.ndarray) -> np.ndarray:
    global LAST_RESULTS
    D = np.ascontiguousarray(np.asarray(D), dtype=np.float32)
    assert D.shape == (B, N, N), D.shape
    shards = D.reshape(N_CORES, N_PAIRS, PAIR, P, FREE)
    # pair tile layout: [128, 2*2048] with matrix m at cols m*2048..
    shards = shards.transpose(0, 1, 3, 2, 4).reshape(N_CORES, N_PAIRS, P, PFREE)
    nc = _get_nc()
    in_maps = [{"d_in": np.ascontiguousarray(shards[i])} for i in range(N_CORES)]
    res = run_bass_kernel_spmd(nc, in_maps, core_ids=list(range(N_CORES)))
    LAST_RESULTS = res
    out = np.stack([res.results[i]["t_out"] for i in range(N_CORES)])
    out = out.reshape(N_CORES, N_PAIRS, P, PAIR, FREE).transpose(0, 1, 3, 2, 4)
    return np.ascontiguousarray(out).reshape(B, N, N)



# revision 5
# speedup vs baseline: 1.1426x; 1.1426x over previous
"""Double-centering kernel for Trainium2 (Bass/Tile), 8-core data parallel.

Computes T = -0.5 * (D - row_mean - col_mean + glob_mean) for
D: [256, 512, 512] f32, sharding the batch dim across 8 NeuronCores
(32 matrices per core, no cross-core communication).

Per-core layout: PAIRS of [512, 512] matrices are viewed as one
[128, 4096] SBUF tile (matrix m in cols m*2048..; partition p holds its
rows 4p..4p+3), so every DMA is one fully contiguous 2 MiB transfer.

Three-stage software pipeline (stage s of pair bp runs at emission
iteration bp+s), one full element pass per engine:
  A (it=bp):   SP    2 MiB load -> in_t            (HWDGE)
               GPSIMD s2 = halves-add(in_t)         (partial col sums)
               PE    C0[m] = ones^T @ s2 halves     (PSUM accumulate x2)
               ACT   rowsums: a_k = 0.5*rowsum_k    (out -> junk PSUM)
  B (it=bp+1): GPSIMD csc[m] = C0/1024 (accum gsum = 256*gmean)
               DVE   rowterm = (a - gsum)/512       (= .5row_mean-.5gmean)
               DVE   in_t = (-0.5*in_t + rowterm) + csc   (affine_then_add,
                                                           in place)
  C (it=bp+2): GPSIMD 2 MiB store <- in_t           (SWDGE)

The gpsimd stream per iteration is s2(it), csc(it-1), store(it-2), so
the store's wait on the DVE pass never blocks the next pair's
reduction.  No v tile: the final DVE pass rewrites in_t in place, which
frees SBUF for a 9-deep in_pool (6-pair load lookahead).
"""

from contextlib import ExitStack

import numpy as np

import concourse.bacc as bacc
import concourse.tile as tile
from concourse import mybir
from concourse.bass_utils import run_bass_kernel_spmd

N_CORES = 8
B = 256
N = 512
B_LOC = B // N_CORES  # 32 matrices per core
PAIR = 2
N_PAIRS = B_LOC // PAIR  # 16 DMA pairs per core
P = 128
CHUNKS = N // P  # 4
FREE = CHUNKS * N  # 2048 elems per partition per matrix
PFREE = PAIR * FREE  # 4096 per pair tile
LOOK = 6  # load lookahead (pairs)

_COMPILED = None
LAST_RESULTS = None  # BassKernelResults of the most recent run (for test harness)


def _build():
    nc = bacc.Bacc("TRN2", target_bir_lowering=False, debug=False)
    d_in = nc.dram_tensor("d_in", [N_PAIRS, P, PFREE], mybir.dt.float32,
                          kind="ExternalInput")
    t_out = nc.dram_tensor("t_out", [N_PAIRS, P, PFREE], mybir.dt.float32,
                           kind="ExternalOutput")
    f32 = mybir.dt.float32

    with tile.TileContext(nc) as tc, ExitStack() as ctx:
        singles = ctx.enter_context(tc.tile_pool(name="singles", bufs=1))
        in_pool = ctx.enter_context(tc.tile_pool(name="in", bufs=9))
        s2_pool = ctx.enter_context(tc.tile_pool(name="s2", bufs=3))
        csc_pool = ctx.enter_context(tc.tile_pool(name="csc", bufs=3))
        a_pool = ctx.enter_context(tc.tile_pool(name="a", bufs=3))
        g_pool = ctx.enter_context(tc.tile_pool(name="g", bufs=3))
        rt_pool = ctx.enter_context(tc.tile_pool(name="rt", bufs=3))
        psum = ctx.enter_context(tc.tile_pool(name="psum", bufs=4, space="PSUM"))
        jpool = ctx.enter_context(tc.tile_pool(name="junk", bufs=1, space="PSUM"))

        ins = [None] * N_PAIRS

        def emit_load(k):
            ins[k] = in_pool.tile([P, PFREE], f32, name="in_t")
            nc.sync.dma_start(out=ins[k][:], in_=d_in[k])

        for k in range(min(LOOK, N_PAIRS)):
            emit_load(k)

        ones_kk = singles.tile([P, P], f32)
        nc.vector.memset(ones_kk[:], 1.0)
        junk = jpool.tile([P, N], f32)  # PSUM discard target for rowsum acts

        st = {}  # per-pair stage-A outputs carried to stage B
        for it in range(N_PAIRS + 2):
            if it < N_PAIRS:
                bp = it
                if it + LOOK < N_PAIRS:
                    emit_load(it + LOOK)
                in_t = ins[bp]

                # Partial column sums: view pair as [128, 2, 2048]; halves
                # add -> s2 [128, 2, 1024] (row c + row c+2 per position).
                in3 = in_t[:].rearrange("p (m f) -> p m f", m=PAIR)
                s2 = s2_pool.tile([P, PAIR, 2 * N], f32)
                nc.gpsimd.tensor_add(out=s2[:], in0=in3[:, :, :2 * N],
                                     in1=in3[:, :, 2 * N:])

                # Column sums broadcast to all partitions: accumulate the two
                # s2 halves through the all-ones matmul into one PSUM bank.
                c0s = []
                for m in range(PAIR):
                    c0 = psum.tile([P, N], f32)
                    nc.tensor.matmul(out=c0[:], lhsT=ones_kk[:],
                                     rhs=s2[:, m, 0:N], start=True, stop=False)
                    nc.tensor.matmul(out=c0[:], lhsT=ones_kk[:],
                                     rhs=s2[:, m, N:2 * N], start=False,
                                     stop=True)
                    c0s.append(c0)

                # Row sums: a_k = 0.5 * rowsum(row 4p+c); the activation's
                # mandatory tensor output goes to a junk PSUM bank.
                a = a_pool.tile([P, PAIR * CHUNKS], f32)
                for m in range(PAIR):
                    for c in range(CHUNKS):
                        sl = slice(m * FREE + c * N, m * FREE + (c + 1) * N)
                        k = m * CHUNKS + c
                        nc.scalar.activation(out=junk[:], in_=in_t[:, sl],
                                             func=mybir.ActivationFunctionType.Copy,
                                             bias=0.0, scale=0.5,
                                             accum_out=a[:, k:k + 1])
                st[bp] = (in_t, c0s, a)

            if 0 <= it - 1 < N_PAIRS:
                bq = it - 1
                in_q, c0s, a = st[bq]

                # csc = 0.5*col_mean (SBUF); gsum = 256*glob_mean.  (ACT:
                # gpsimd cannot read PSUM.)
                csc = csc_pool.tile([P, PAIR, N], f32)
                gsum = g_pool.tile([P, PAIR], f32)
                for m in range(PAIR):
                    nc.scalar.activation(out=csc[:, m, :], in_=c0s[m][:],
                                         func=mybir.ActivationFunctionType.Copy,
                                         bias=0.0, scale=1.0 / 1024.0,
                                         accum_out=gsum[:, m:m + 1])

                # rowterm = (a - gsum)/512 = 0.5*row_mean - 0.5*glob_mean.
                rowterm = rt_pool.tile([P, PAIR * CHUNKS], f32)
                for m in range(PAIR):
                    ksl = slice(m * CHUNKS, (m + 1) * CHUNKS)
                    nc.vector.tensor_scalar(out=rowterm[:, ksl], in0=a[:, ksl],
                                            scalar1=gsum[:, m:m + 1],
                                            scalar2=1.0 / 512.0,
                                            op0=mybir.AluOpType.subtract,
                                            op1=mybir.AluOpType.mult)

                # out_c = (-0.5*D_c + rowterm_c) + csc, one DVE op per chunk,
                # in place over in_t.
                for m in range(PAIR):
                    for c in range(CHUNKS):
                        sl = slice(m * FREE + c * N, m * FREE + (c + 1) * N)
                        k = m * CHUNKS + c
                        nc.vector.affine_then_add(out=in_q[:, sl],
                                                  in0=in_q[:, sl],
                                                  in1=csc[:, m, :],
                                                  scale=-0.5,
                                                  bias=rowterm[:, k:k + 1])

            if 0 <= it - 2 < N_PAIRS:
                br = it - 2
                nc.gpsimd.dma_start(out=t_out[br], in_=st[br][0][:])

    nc.compile()
    return nc


def _get_nc():
    global _COMPILED
    if _COMPILED is None:
        _COMPILED = _build()
    return _COMPILED


def kernel(D: np.ndarray) -> np.ndarray:
    global LAST_RESULTS
    D = np.ascontiguousarray(np.asarray(D), dtype=np.float32)
    assert D.shape == (B, N, N), D.shape
    shards = D.reshape(N_CORES, N_PAIRS, PAIR, P, FREE)
    # pair tile layout: [128, 2*2048] with matrix m at cols m*2048..
    shards = shards.transpose(0, 1, 3, 2, 4).reshape(N_CORES, N_PAIRS, P, PFREE)
    nc = _get_nc()
    in_maps = [{"d_in": np.ascontiguousarray(shards[i])} for i in range(N_CORES)]
    res = run_bass_kernel_spmd(nc, in_maps, core_ids=list(range(N_CORES)))
    LAST_RESULTS = res
    out = np.stack([res.results[i]["t_out"] for i in range(N_CORES)])
    out = out.reshape(N_CORES, N_PAIRS, P, PAIR, FREE).transpose(0, 1, 3, 2, 4)
    return np.ascontiguousarray(out).reshape(B, N, N)
